# revision 65
# baseline (speedup 1.0000x reference)
"""Trainium2 Bass kernel for nn_Net_int_12421045420311 (GNN message passing).

Model (see problem reference):
  out = relu(x @ Wn + bn)                         [N, 64]
  ea  = relu(edge_attr @ We + be)                 [E, 12]
  Wedge = (relu(ea @ W1 + b1) @ W2 + b2)          [E, 64, 64]
  3x: msg_e = out[src_e] @ Wedge_e ; agg = scatter_mean(msg, dst) ;
      h = GRU(relu(agg + conv_b), h) ; out = h
  pair readout over 65536 node pairs + train-mode batchnorm + linear head.

Distribution over 8 cores: edges sorted by dst and sharded by dst range
(N/8 nodes per core) so every core owns complete scatter sums for its
nodes; the node table is AllGathered between iterations (bf16); pairs are
sharded for the readout and batchnorm stats are AllReduced.

v2 design notes:
 - Wedge stored once to DRAM in bf16 with (o,i)-permuted columns so the
   per-edge contraction sum_i A[e,i]*Wedge[e,(o,i)] runs as one packed-
   bf16 tensor_tensor (2x DVE mode) + one unit-stride tensor_reduce.
 - one-hot scatter matrices precomputed on host ({0,1} in bf16, exact),
   SBUF-resident across iterations; 1/deg applied post-scatter in fp32.
 - scatter-sums via PE matmuls with bf16 operands.
 - node tables, gathers, messages, collectives in bf16; GRU and
   batchnorm statistics kept in fp32.
 - b2 (edge-MLP bias) via associativity:
   sum_e onehot_e*(A_e@b2r) = (sum_e onehot_e*A_e) @ b2r.
 - gathers via the GPSIMD dma_gather extended instruction, issued
   per 512-node tile so they overlap with compute of the previous tile.
"""

import numpy as np
import ml_dtypes

import concourse.bass as bass
import concourse.mybir as mybir
import concourse.tile as tile
import bass_rust as _bass_rust
from concourse import bass_utils
from concourse.bass import ts, ds
from concourse.library_config import all_libraries, standard
from concourse.masks import make_identity

F32 = mybir.dt.float32
BF16 = mybir.dt.bfloat16
I16 = mybir.dt.int16
AF = mybir.ActivationFunctionType
OP = mybir.AluOpType

N_CORES = 8
DIM = 64
TDIM = 128            # node-table row padded to 256B for dma_gather
GATHER_PIECE = 1024   # max idxs per dma_gather (descriptor-ring capacity)
HID = 128
EDIM = 12
EPS = 1e-5
BF = ml_dtypes.bfloat16


class Cfg:
    def __init__(self, n_nodes, n_edges, n_pairs, ch_per_tile,
                 n_iters=3, readout=True, it_depth=4, ro_depth=4,
                 use_b2=True):
        self.n = n_nodes
        self.e = n_edges
        self.p = n_pairs
        self.n_loc = n_nodes // N_CORES
        self.p_loc = n_pairs // N_CORES
        self.tile_nodes = min(512, self.n_loc)
        self.n_tiles = self.n_loc // self.tile_nodes
        self.subt = self.tile_nodes // 128
        self.ch_per_tile = ch_per_tile
        self.chunks = self.n_tiles * ch_per_tile
        self.e_pad = self.chunks * 128
        self.p_chunks = self.p_loc // 128
        self.n_iters = n_iters
        self.readout = readout
        self.it_depth = it_depth
        self.ro_depth = ro_depth
        self.use_b2 = use_b2
        self.key = (n_nodes, n_edges, n_pairs, ch_per_tile, n_iters, readout,
                    it_depth, ro_depth, use_b2)


def _ap(base, dims, off=0):
    return bass.AP(base.tensor, base.offset + off, [list(d) for d in dims])


def _legalize_waits(nc, keep=1, keep_extended=0):
    """Split multi-wait instructions into preceding single-wait NoOps.

    This walrus build's setupSyncWait accepts at most one sync wait per
    instruction, while Tile attaches one wait per producer semaphore.
    Waits execute on the engine sequencer in program order, so hoisting
    them onto NoOps preserves semantics.
    """
    n = 0
    ext = ("DMAGatherAnt", "DMAScatterAddAnt", "KVWritebackAnt",
           "PagedWritebackAnt")
    for f in nc.m.functions:
        for bb in f.blocks:
            out = []
            for ins in bb.instructions:
                si = ins.sync_info
                k = keep_extended if type(ins).__name__.removeprefix("Inst") in ext else keep
                if si is not None and si.on_wait is not None and len(si.on_wait) > k:
                    waits = list(si.on_wait)
                    for w in (waits[:-k] if k else waits):
                        nop = mybir.InstNoOp(name=f"WS-{n}", text_hint="waitsplit")
                        n += 1
                        nop.engine = ins.engine
                        nop.sync_info = mybir.SyncInfo(on_wait=[w], on_update=[])
                        nc.register_instruction(nop, overwrite=True)
                        out.append(nop)
                    ins.sync_info = mybir.SyncInfo(
                        on_wait=(waits[-k:] if k else []),
                        on_update=list(si.on_update))
                out.append(ins)
            bb.instructions = out
    return n


def _insert_library_loads(nc):
    """bacc.insert_library_loads equivalent for plain Bass: dma_gather &
    friends need the 'mlp' GPSIMD ucode library loaded."""
    mask = {}
    for lib in all_libraries:
        for t in lib.instructions:
            mask[t] = mask.get(t, 0) | (1 << lib.index)
    _bass_rust.insert_library_loads(nc, mask, len(all_libraries), standard.index)


def _gather_split(nc, out_tile, table, idx_sb, total, col0=0, queue=0):
    """dma_gather in <=GATHER_PIECE chunks (descriptor carveout is ~1024
    descs; one big gather would deadlock awaiting ring space)."""
    for off in range(0, total, GATHER_PIECE):
        w = min(GATHER_PIECE, total - off)
        nc.gpsimd.dma_gather(
            out_tile[:, off // 128:(off + w) // 128, :], table[:],
            idx_sb[:, col0 + off // 16:col0 + (off + w) // 16], w, w, TDIM,
            queue_num=queue)


def build_nc(c: Cfg):
    nc = bass.Bass(num_swdge_queues=4)
    WSQ = DIM * DIM
    NSUB = c.n_tiles * c.subt          # 128-node subtiles per core

    # ---------------- I/O ----------------
    xTl = nc.dram_tensor("xTl", [9, c.n_loc], F32, kind="ExternalInput")
    Wn_ext = nc.dram_tensor("Wn_ext", [9, DIM], F32, kind="ExternalInput")
    eaT = nc.dram_tensor("eaT", [19, c.e_pad], F32, kind="ExternalInput")
    We_in = nc.dram_tensor("We_in", [19, EDIM], F32, kind="ExternalInput")
    W1_in = nc.dram_tensor("W1_in", [EDIM, HID], F32, kind="ExternalInput")
    be_c = nc.dram_tensor("be_c", [EDIM, 1], F32, kind="ExternalInput")
    b1_c = nc.dram_tensor("b1_c", [HID, 1], F32, kind="ExternalInput")
    W2 = nc.dram_tensor("W2", [HID, WSQ], F32, kind="ExternalInput")
    b2p = nc.dram_tensor("b2p", [1, WSQ], BF16, kind="ExternalInput")
    conv_bc = nc.dram_tensor("conv_bc", [128, DIM], F32, kind="ExternalInput")
    gidx = nc.dram_tensor("gidx", [128, c.e_pad // 16], I16, kind="ExternalInput")
    ohd = nc.dram_tensor("ohd", [128, c.chunks * c.tile_nodes], BF16,
                         kind="ExternalInput")
    rdegc = nc.dram_tensor("rdegc", [128, NSUB], F32, kind="ExternalInput")
    WihT = nc.dram_tensor("WihT", [DIM, 3 * DIM], BF16, kind="ExternalInput")
    WhhT = nc.dram_tensor("WhhT", [DIM, 3 * DIM], BF16, kind="ExternalInput")
    b_r = nc.dram_tensor("b_r", [DIM, 1], F32, kind="ExternalInput")
    b_z = nc.dram_tensor("b_z", [DIM, 1], F32, kind="ExternalInput")
    bihn = nc.dram_tensor("bihn", [DIM, 1], F32, kind="ExternalInput")
    bhhn = nc.dram_tensor("bhhn", [DIM, 1], F32, kind="ExternalInput")
    idx30 = nc.dram_tensor("idx30", [128, c.p_loc // 16], I16, kind="ExternalInput")
    idx31 = nc.dram_tensor("idx31", [128, c.p_loc // 16], I16, kind="ExternalInput")
    ea3T = nc.dram_tensor("ea3T", [8, c.p_loc], BF16, kind="ExternalInput")
    Wlw = nc.dram_tensor("Wlw", [8, 3 * DIM], F32, kind="ExternalInput")
    Wlb = nc.dram_tensor("Wlb", [8, 1], F32, kind="ExternalInput")
    gamma = nc.dram_tensor("gamma", [1, 3 * DIM], F32, kind="ExternalInput")
    beta = nc.dram_tensor("beta", [1, 3 * DIM], F32, kind="ExternalInput")
    epsv = nc.dram_tensor("epsv", [1, 3 * DIM], F32, kind="ExternalInput")
    y = nc.dram_tensor("y", [128, c.p_chunks], F32, kind="ExternalOutput")

    # internal DRAM
    cc_in = [nc.dram_tensor(f"cc_in{i}", [c.n_loc, TDIM], BF16)
             for i in range(4)]
    cc_out = [nc.dram_tensor(f"cc_out{i}", [c.n, TDIM], BF16,
                         addr_space="Shared") for i in range(4)]
    tab0 = cc_out[3]
    st_in = nc.dram_tensor("st_in", [1, 6 * DIM], F32)
    st_out = nc.dram_tensor("st_out", [1, 6 * DIM], F32, addr_space="Shared")

    rgroups = [list(range(N_CORES))]

    with tile.TileContext(nc) as tc:
      with (
          tc.tile_pool(name="persist", bufs=1) as pp,
      ):
        # ------------- persistent small tensors -------------
        ident = pp.tile([128, 128], F32)
        make_identity(nc, ident[:])

        Wn_sb = pp.tile([9, DIM], F32)
        nc.sync.dma_start(Wn_sb[:], Wn_ext[:])
        W2_sb = pp.tile([HID, WSQ], BF16)
        nc.gpsimd.dma_start(W2_sb[:], W2[:])  # SWDGE cast f32->bf16
        h1T = pp.tile([HID, c.e_pad], BF16)
        if c.use_b2:
            b2p_sb = pp.tile([1, WSQ], BF16)
            nc.sync.dma_start(b2p_sb[:], b2p[:])
            ones1 = pp.tile([1, 128], BF16)
            nc.vector.memset(ones1[:], 1.0)
        convb_sb = pp.tile([128, DIM], F32)
        nc.sync.dma_start(convb_sb[:], conv_bc[:])
        WihT_sb = pp.tile([DIM, 3 * DIM], BF16)
        nc.sync.dma_start(WihT_sb[:], WihT[:])
        WhhT_sb = pp.tile([DIM, 3 * DIM], BF16)
        nc.sync.dma_start(WhhT_sb[:], WhhT[:])
        br_sb = pp.tile([DIM, 1], F32)
        nc.sync.dma_start(br_sb[:], b_r[:])
        bz_sb = pp.tile([DIM, 1], F32)
        nc.sync.dma_start(bz_sb[:], b_z[:])
        bihn_sb = pp.tile([DIM, 1], F32)
        nc.sync.dma_start(bihn_sb[:], bihn[:])
        bhhn_sb = pp.tile([DIM, 1], F32)
        nc.sync.dma_start(bhhn_sb[:], bhhn[:])
        gidx_sb = pp.tile([128, c.e_pad // 16], I16)
        nc.sync.dma_start(gidx_sb[:], gidx[:])
        oh_sb = pp.tile([128, c.chunks, c.tile_nodes], BF16)
        nc.sync.dma_start(
            oh_sb[:], ohd.rearrange("p (ch t) -> p ch t", t=c.tile_nodes))
        rdeg_sb = pp.tile([128, NSUB], F32)
        nc.sync.dma_start(rdeg_sb[:], rdegc[:])

        hTf = pp.tile([DIM, c.n_loc], F32)      # node state (transposed, fp32)
        hT = pp.tile([DIM, c.n_loc], BF16)      # bf16 copy for matmul moving

        # ------------- phase 0: initial node embeddings -------------
        # local transposed state h0T = relu(Wn_ext.T @ x_extT_local);
        # rows via PE transpose; AllGather replicates the full table.
        with (
            tc.tile_pool(name="p0", bufs=2) as sp,
            tc.tile_pool(name="p0ps", bufs=2, space="PSUM") as ps2,
        ):
            xl = sp.tile([9, c.n_loc], F32, tag="xl")
            nc.sync.dma_start(xl[:], xTl[:])
            for j in range(0, c.n_loc, 512):
                w = min(512, c.n_loc - j)
                ph = ps2.tile([DIM, 512], F32, tag="p0h")
                nc.tensor.matmul(ph[:, :w], Wn_sb[:], xl[:, ds(j, w)],
                                 start=True, stop=True)
                nc.scalar.activation(hTf[:, ds(j, w)], ph[:, :w], AF.Relu)
                nc.vector.tensor_copy(hT[:, ds(j, w)], hTf[:, ds(j, w)])
            rows0 = sp.tile([128, c.n_loc // 128, TDIM], BF16, tag="rows0")
            for g in range(c.n_loc // 128):
                ptg = ps2.tile([128, 128], F32, tag="p0t")
                nc.tensor.transpose(ptg[:, :DIM], hTf[:, ts(g, 128)],
                                    ident[:DIM, :DIM])
                nc.vector.tensor_copy(rows0[:, g, :DIM], ptg[:, :DIM])
            nc.sync.dma_start(
                cc_in[3].rearrange("(g p) d -> p g d", p=128), rows0[:])
            nc.gpsimd.collective_compute(
                "AllGather", OP.bypass, replica_groups=rgroups,
                ins=[cc_in[3].ap().opt()], outs=[cc_out[3].ap().opt()])

        # ------------- phase A: edge MLP -> h1T (wedge recomputed per iter) --
        with (
            tc.tile_pool(name="pa", bufs=3) as sp,
            tc.tile_pool(name="pah", bufs=1) as hp,
        ):
            W1_sb = hp.tile([EDIM, HID], F32)
            nc.sync.dma_start(W1_sb[:], W1_in[:])
            We_sb = hp.tile([19, EDIM], F32)
            nc.sync.dma_start(We_sb[:], We_in[:])
            be_sb = hp.tile([EDIM, 1], F32)
            nc.sync.dma_start(be_sb[:], be_c[:])
            b1_sb = hp.tile([HID, 1], F32)
            nc.sync.dma_start(b1_sb[:], b1_c[:])
            ea_sb = hp.tile([EDIM, c.e_pad], F32)
            with tc.tile_pool(name="paps1", bufs=2, space="PSUM") as psA:
                for j in range(0, c.e_pad, 512):
                    w = min(512, c.e_pad - j)
                    et = sp.tile([19, 512], F32, tag="et")
                    nc.sync.dma_start(et[:, :w], eaT[:, ds(j, w)])
                    pe = psA.tile([EDIM, 512], F32, tag="pe")
                    nc.tensor.matmul(pe[:, :w], We_sb[:], et[:, :w],
                                     start=True, stop=True)
                    nc.scalar.activation(ea_sb[:, ds(j, w)], pe[:, :w],
                                         AF.Relu, bias=be_sb[:])
                for j in range(0, c.e_pad, 512):
                    w = min(512, c.e_pad - j)
                    ph1 = psA.tile([HID, 512], F32, tag="ph1")
                    nc.tensor.matmul(ph1[:, :w], W1_sb[:], ea_sb[:, ds(j, w)],
                                     start=True, stop=True)
                    nc.scalar.activation(h1T[:, ds(j, w)], ph1[:, :w], AF.Relu,
                                         bias=b1_sb[:])

        # ------------- 3 message-passing iterations -------------
        for it in range(c.n_iters):
            tab_prev = tab0 if it == 0 else cc_out[it - 1]
            with (
                tc.tile_pool(name=f"it{it}", bufs=1) as ip,
                tc.tile_pool(name=f"ita{it}", bufs=4) as ap,
                tc.tile_pool(name=f"itw{it}", bufs=4) as wp,
                tc.tile_pool(name=f"its{it}", bufs=4) as sp,
                tc.tile_pool(name=f"itpw{it}", bufs=2, space="PSUM") as psW,
                tc.tile_pool(name=f"itps{it}", bufs=1, space="PSUM") as psI,
                tc.tile_pool(name=f"itp1{it}", bufs=1, space="PSUM") as psM,
                tc.tile_pool(name=f"itpg{it}", bufs=1, space="PSUM") as psG,
            ):
                mT = ip.tile([DIM, c.n_loc], BF16)
                if c.it_depth < 2:
                    nc.vector.tensor_copy(mT[:, :DIM], hT[:, :DIM])
                    continue
                te = c.ch_per_tile * 128     # edges per tile (padded)
                for t in range(c.n_tiles):
                    At = ap.tile([128, c.ch_per_tile, TDIM], BF16, tag="at")
                    Mt = ap.tile([128, c.ch_per_tile, DIM], BF16, tag="mt")
                    _gather_split(nc, At, tab_prev, gidx_sb, te,
                                  col0=t * te // 16, queue=t % 4)
                    # per-chunk contraction msg_e = sum_i A[e,i]*W[e,(o,i)];
                    # Wedge recomputed on PE into PSUM quarters, staged to
                    # SBUF in bf16 by the scalar engine
                    for k in range(c.ch_per_tile):
                        ch = t * c.ch_per_tile + k
                        wt = wp.tile([128, WSQ], BF16, tag="wld")
                        for q in range(4):
                            pw = psW.tile([128, WSQ // 4], F32, tag="pw")
                            for hf in range(2):
                                nc.tensor.matmul(
                                    pw[:, ts(hf, 512)], h1T[:, ts(ch, 128)],
                                    W2_sb[:, ds(q * 1024 + hf * 512, 512)],
                                    start=True, stop=not c.use_b2)
                                if c.use_b2:
                                    nc.tensor.matmul(
                                        pw[:, ts(hf, 512)], ones1[:],
                                        b2p_sb[:, ds(q * 1024 + hf * 512, 512)],
                                        start=False, stop=True)
                            nc.scalar.copy(wt[:, ts(q, WSQ // 4)], pw[:])
                        prod = wp.tile([128, WSQ], BF16, tag="prod")
                        nc.vector.tensor_tensor(
                            _ap(prod[:], [[WSQ, 128], [DIM, DIM], [1, DIM]]),
                            _ap(wt[:], [[WSQ, 128], [DIM, DIM], [1, DIM]]),
                            _ap(At[:], [[c.ch_per_tile * TDIM, 128], [0, DIM],
                                        [1, DIM]], off=k * TDIM),
                            OP.mult)
                        # tree-reduce over i (innermost) with packed-bf16 adds
                        for r in (32, 16, 8, 4, 2):
                            nc.vector.tensor_tensor(
                                _ap(prod[:], [[WSQ, 128], [DIM, DIM], [1, r]]),
                                _ap(prod[:], [[WSQ, 128], [DIM, DIM], [1, r]]),
                                _ap(prod[:], [[WSQ, 128], [DIM, DIM], [1, r]],
                                    off=r),
                                OP.add)
                        nc.vector.tensor_tensor(
                            Mt[:, k, :],
                            _ap(prod[:], [[WSQ, 128], [DIM, DIM]]),
                            _ap(prod[:], [[WSQ, 128], [DIM, DIM]], off=1),
                            OP.add)
                    # scatter-sums over the tile's 4 node subtiles
                    pm = psM.tile([128, c.subt * DIM], F32, tag="pm")
                    for s in range(c.subt):
                        for k in range(c.ch_per_tile):
                            ch = t * c.ch_per_tile + k
                            nc.tensor.matmul(
                                pm[:, ts(s, DIM)], oh_sb[:, ch, ds(s * 128, 128)],
                                Mt[:, k, :], start=(k == 0),
                                stop=(k == c.ch_per_tile - 1))
                    # m = relu(pm/deg + conv_b); transpose into mT
                    mrow = sp.tile([128, c.subt * DIM], F32, tag="mrow")
                    for s in range(c.subt):
                        nc.vector.tensor_scalar(
                            mrow[:, ts(s, DIM)], pm[:, ts(s, DIM)],
                            rdeg_sb[:, ds(t * c.subt + s, 1)], None, OP.mult)
                    nc.vector.tensor_tensor(
                        mrow[:], mrow[:],
                        _ap(convb_sb[:], [[DIM, 128], [0, c.subt], [1, DIM]]),
                        OP.add)
                    nc.scalar.activation(mrow[:], mrow[:], AF.Relu)
                    for s in range(c.subt):
                        ptm = psI.tile([128, 128], F32, tag="ptx")
                        nc.tensor.transpose(ptm[:DIM, :], mrow[:, ts(s, DIM)],
                                            ident[:])
                        nc.vector.tensor_copy(
                            mT[:, ds(t * c.tile_nodes + s * 128, 128)],
                            ptm[:DIM, :])

                    # ---- GRU + h-rows for this tile's 512-node piece ----
                    if c.it_depth < 3:
                        continue
                    j = t * c.tile_nodes
                    w = c.tile_nodes
                    pgA = psG.tile([128, 512], F32, tag="pgA")
                    pgB = psG.tile([128, 512], F32, tag="pgB")
                    pr = pgA[0:DIM, :]
                    pz = pgA[DIM:2 * DIM, :]
                    pxn = pgB[0:DIM, :]
                    phn = pgB[DIM:2 * DIM, :]
                    nc.tensor.matmul(pr[:, :w], WihT_sb[:, 0:DIM],
                                     mT[:, ds(j, w)], start=True, stop=False)
                    nc.tensor.matmul(pr[:, :w], WhhT_sb[:, 0:DIM],
                                     hT[:, ds(j, w)], start=False, stop=True)
                    nc.tensor.matmul(pz[:, :w], WihT_sb[:, DIM:2 * DIM],
                                     mT[:, ds(j, w)], start=True, stop=False)
                    nc.tensor.matmul(pz[:, :w], WhhT_sb[:, DIM:2 * DIM],
                                     hT[:, ds(j, w)], start=False, stop=True)
                    nc.tensor.matmul(pxn[:, :w], WihT_sb[:, 2 * DIM:],
                                     mT[:, ds(j, w)], start=True, stop=True)
                    nc.tensor.matmul(phn[:, :w], WhhT_sb[:, 2 * DIM:],
                                     hT[:, ds(j, w)], start=True, stop=True)
                    r_sb = sp.tile([DIM, 512], BF16, tag="r")
                    nc.scalar.activation(r_sb[:, :w], pr[:, :w], AF.Sigmoid,
                                         bias=br_sb[:])
                    z_sb = sp.tile([DIM, 512], BF16, tag="z")
                    nc.scalar.activation(z_sb[:, :w], pz[:, :w], AF.Sigmoid,
                                         bias=bz_sb[:])
                    ghn = sp.tile([DIM, 512], BF16, tag="ghn")
                    nc.scalar.activation(ghn[:, :w], phn[:, :w], AF.Identity,
                                         bias=bhhn_sb[:])
                    nc.vector.tensor_tensor(ghn[:, :w], r_sb[:, :w], ghn[:, :w],
                                            OP.mult)
                    s_sb = sp.tile([DIM, 512], F32, tag="s")
                    nc.vector.tensor_tensor(s_sb[:, :w], pxn[:, :w], ghn[:, :w],
                                            OP.add)
                    n_sb = sp.tile([DIM, 512], BF16, tag="n")
                    nc.scalar.activation(n_sb[:, :w], s_sb[:, :w], AF.Tanh,
                                         bias=bihn_sb[:])
                    d_sb = sp.tile([DIM, 512], BF16, tag="d")
                    nc.vector.tensor_tensor(d_sb[:, :w], hT[:, ds(j, w)],
                                            n_sb[:, :w], OP.subtract)
                    nc.vector.tensor_tensor(d_sb[:, :w], z_sb[:, :w], d_sb[:, :w],
                                            OP.mult)
                    nc.vector.tensor_tensor(hT[:, ds(j, w)], n_sb[:, :w],
                                            d_sb[:, :w], OP.add)
                    nc.vector.tensor_tensor(hTf[:, ds(j, w)], n_sb[:, :w],
                                            d_sb[:, :w], OP.add)
                    if c.it_depth < 4:
                        continue
                    gpt = c.tile_nodes // 128
                    rows = ap.tile([128, gpt, TDIM], BF16, tag="rows")
                    for g in range(gpt):
                        ptg = psI.tile([128, 128], F32, tag="ptx")
                        nc.tensor.transpose(ptg[:, :DIM],
                                            hTf[:, ds(j + g * 128, 128)],
                                            ident[:DIM, :DIM])
                        nc.vector.tensor_copy(rows[:, g, :DIM], ptg[:, :DIM])
                    nc.sync.dma_start(
                        cc_in[it].rearrange("(t g p) d -> t p g d",
                                            p=128, g=gpt)[t], rows[:])

                # ---- AllGather the updated node rows -> cc_out[it] ----
                if c.it_depth < 4:
                    continue
                nc.gpsimd.collective_compute(
                    "AllGather", OP.bypass, replica_groups=rgroups,
                    ins=[cc_in[it].ap().opt()], outs=[cc_out[it].ap().opt()])

        # ------------- readout -------------
        G3 = 3 * DIM
        PIT = G3 + 2                  # [mean|prod|diff2|ones|zero] per pair
        if not c.readout:
            dbg = pp.tile([128, 8], F32)
            if c.n_iters and c.it_depth >= 4:
                nc.sync.dma_start(dbg[:, :1],
                                  cc_out[c.n_iters - 1][:128, :1])
            else:
                nc.sync.dma_start(dbg[:, :1], tab0[:128, :1])
            nc.vector.tensor_copy(dbg[:, 1:2], dbg[:, :1])
            yz = pp.tile([128, c.p_chunks], F32)
            nc.vector.memset(yz[:], 0.0)
            nc.vector.tensor_tensor(yz[:, :1], yz[:, :1], dbg[:, 1:2], OP.add)
            nc.sync.dma_start(y[:], yz[:])
        if c.readout:
          with (
              tc.tile_pool(name="ro", bufs=1) as rp,
              tc.tile_pool(name="ros", bufs=2) as sp,
              tc.tile_pool(name="rops", bufs=2, space="PSUM") as psR,
              tc.tile_pool(name="rop1", bufs=1, space="PSUM") as psS,
          ):
              tab_fin = cc_out[c.n_iters - 1] if c.n_iters else tab0
              i30 = rp.tile([128, c.p_loc // 16], I16)
              nc.sync.dma_start(i30[:], idx30[:])
              i31 = rp.tile([128, c.p_loc // 16], I16)
              nc.sync.dma_start(i31[:], idx31[:])
              ea3_sb = rp.tile([8, c.p_loc], BF16)
              nc.sync.dma_start(ea3_sb[:], ea3T[:])

              yh = rp.tile([128, c.p_chunks, PIT], BF16)  # [sum|prod|diff2|1|0]
              nc.vector.memset(yh[:, :, G3:G3 + 1], 1.0)
              nc.vector.memset(yh[:, :, G3 + 1:G3 + 2], 0.0)
              ysq = rp.tile([128, c.p_chunks, G3], BF16)
              ones_l = rp.tile([128, 1], BF16)
              nc.vector.memset(ones_l[:], 1.0)
              pst = psS.tile([1, G3], F32, tag="ps_s")
              psq = psS.tile([1, G3], F32, tag="ps_q")
              # gather + pair features + stats, one 1024-pair piece at a time
              PC = GATHER_PIECE // 128
              for pc in range(c.p_chunks // PC):
                  t0p = sp.tile([128, PC, TDIM], BF16, tag="t0p")
                  t1p = sp.tile([128, PC, TDIM], BF16, tag="t1p")
                  icol = pc * GATHER_PIECE // 16
                  q = pc % 4
                  _gather_split(nc, t0p, tab_fin, i30, GATHER_PIECE,
                                col0=icol, queue=q)
                  _gather_split(nc, t1p, tab_fin, i31, GATHER_PIECE,
                                col0=icol, queue=q)
                  pcD = [[PC * TDIM, 128], [TDIM, PC], [1, DIM]]
                  yhD = [[c.p_chunks * PIT, 128], [PIT, PC], [1, DIM]]
                  yb = pc * PC * PIT
                  nc.vector.tensor_tensor(
                      _ap(yh[:], yhD, off=yb), _ap(t0p[:], pcD),
                      _ap(t1p[:], pcD), OP.add)
                  nc.vector.tensor_tensor(
                      _ap(yh[:], yhD, off=yb + DIM), _ap(t0p[:], pcD),
                      _ap(t1p[:], pcD), OP.mult)
                  nc.vector.tensor_tensor(
                      _ap(yh[:], yhD, off=yb + 2 * DIM), _ap(t0p[:], pcD),
                      _ap(t1p[:], pcD), OP.subtract)
                  nc.scalar.square(
                      _ap(yh[:], yhD, off=yb + 2 * DIM),
                      _ap(yh[:], yhD, off=yb + 2 * DIM))
                  nc.scalar.square(
                      ysq[:, pc * PC:(pc + 1) * PC, :],
                      _ap(yh[:], [[c.p_chunks * PIT, 128], [PIT, PC],
                                  [1, G3]], off=yb))
                  for g in range(pc * PC, (pc + 1) * PC):
                      nc.tensor.matmul(
                          pst[:], ones_l[:],
                          _ap(yh[:], [[c.p_chunks * PIT, 128], [1, G3]],
                              off=g * PIT),
                          start=(g == 0), stop=(g == c.p_chunks - 1))
                      nc.tensor.matmul(
                          psq[:], ones_l[:], ysq[:, g, :],
                          start=(g == 0), stop=(g == c.p_chunks - 1))
              st_sb = sp.tile([1, 2 * G3], F32, tag="st")
              nc.vector.tensor_copy(st_sb[:, :G3], pst[:])
              nc.scalar.copy(st_sb[:, G3:], psq[:])
              nc.sync.dma_start(st_in[:], st_sb[:])
              nc.gpsimd.collective_compute(
                  "AllReduce", OP.add, replica_groups=rgroups,
                  ins=[st_in.ap().opt()], outs=[st_out.ap().opt()])
              stg = sp.tile([1, 2 * G3], F32, tag="stg")
              nc.sync.dma_start(stg[:], st_out[:])

              # mu, var, rstd (with one Newton step), g' = rstd*gamma,
              # b' = beta - mu*g'
              gam_sb = sp.tile([1, G3], F32, tag="gam")
              nc.sync.dma_start(gam_sb[:], gamma[:])
              bet_sb = sp.tile([1, G3], F32, tag="bet")
              nc.sync.dma_start(bet_sb[:], beta[:])
              mu = sp.tile([1, G3], F32, tag="mu")
              nc.vector.tensor_scalar_mul(mu[:], stg[:, :G3], 1.0 / c.p)
              var = sp.tile([1, G3], F32, tag="var")
              nc.vector.tensor_scalar_mul(var[:], stg[:, G3:], 1.0 / c.p)
              musq = sp.tile([1, G3], F32, tag="musq")
              nc.vector.tensor_tensor(musq[:], mu[:], mu[:], OP.mult)
              nc.vector.tensor_tensor(var[:], var[:], musq[:], OP.subtract)
              epsv_sb = sp.tile([1, G3], F32, tag="epsv")
              nc.sync.dma_start(epsv_sb[:], epsv[:])
              ve = sp.tile([1, G3], F32, tag="ve")
              nc.vector.tensor_tensor(ve[:], var[:], epsv_sb[:], OP.add)
              sq = sp.tile([1, G3], F32, tag="sq")
              nc.scalar.activation(sq[:], ve[:], AF.Sqrt)
              r0 = sp.tile([1, G3], F32, tag="r0")
              nc.vector.reciprocal(r0[:], sq[:])
              # Newton: r1 = r0*(1.5 - 0.5*(var+eps)*r0^2)
              t_ = sp.tile([1, G3], F32, tag="t_")
              nc.vector.tensor_tensor(t_[:], r0[:], r0[:], OP.mult)
              nc.vector.tensor_tensor(t_[:], t_[:], ve[:], OP.mult)
              nc.vector.tensor_scalar(t_[:], t_[:], -0.5, 1.5, OP.mult, OP.add)
              nc.vector.tensor_tensor(r0[:], r0[:], t_[:], OP.mult)
              gp = sp.tile([1, G3], F32, tag="gp")
              nc.vector.tensor_tensor(gp[:], r0[:], gam_sb[:], OP.mult)
              bp = sp.tile([1, G3], F32, tag="bp")
              nc.vector.tensor_tensor(bp[:], mu[:], gp[:], OP.mult)
              nc.vector.tensor_tensor(bp[:], bet_sb[:], bp[:], OP.subtract)

              gpb = sp.tile([8, G3], F32, tag="gpb")
              nc.gpsimd.partition_broadcast(gpb[:], gp[:])
              bpb = sp.tile([8, G3], F32, tag="bpb")
              nc.gpsimd.partition_broadcast(bpb[:], bp[:])
              Wlw_sb = sp.tile([8, G3], F32, tag="wlw")
              nc.sync.dma_start(Wlw_sb[:], Wlw[:])
              Wlb_sb = sp.tile([8, 1], F32, tag="wlb")
              nc.sync.dma_start(Wlb_sb[:], Wlb[:])
              comb = rp.tile([8, PIT], BF16)
              nc.vector.memset(comb[:, G3 + 1:], 0.0)
              nc.vector.tensor_tensor(comb[:, :G3], Wlw_sb[:], gpb[:], OP.mult)
              vb = sp.tile([8, G3], F32, tag="vb")
              nc.vector.tensor_tensor(vb[:], Wlw_sb[:], bpb[:], OP.mult)
              vbr = sp.tile([8, 1], F32, tag="vbr")
              nc.vector.tensor_reduce(vbr[:], vb[:], mybir.AxisListType.X, OP.add)
              nc.vector.tensor_tensor(comb[:, G3:G3 + 1], Wlb_sb[:], vbr[:],
                                      OP.add)

              y_sb = rp.tile([128, c.p_chunks], F32)
              for g in range(c.p_chunks):
                  pw = psR.tile([128, PIT], F32, tag="pw")
                  nc.tensor.matmul(pw[:], ea3_sb[:, ts(g, 128)], comb[:],
                                   start=True, stop=True)
                  scr = sp.tile([128, PIT], F32, tag="scr")
                  nc.vector.scalar_tensor_tensor(
                      scr[:],
                      _ap(yh[:], [[c.p_chunks * PIT, 128], [1, PIT]],
                          off=g * PIT),
                      1.0, pw[:], OP.mult, OP.mult,
                      accum_out=y_sb[:, ds(g, 1)])
              nc.sync.dma_start(y[:], y_sb[:])

    _insert_library_loads(nc)
    mybir.codegen_inst_isa_subclasses(nc)
    _legalize_waits(nc)
    return nc


_NC_CACHE = {}


def _get_nc(cfg: Cfg):
    nc = _NC_CACHE.get(cfg.key)
    if nc is None:
        nc = build_nc(cfg)
        _NC_CACHE[cfg.key] = nc
    return nc


def _wrap16(a):
    """int token array [M] -> [128, M//16] int16 gather-index layout."""
    m = a.shape[0]
    return np.ascontiguousarray(
        np.tile(a.astype(np.int16).reshape(m // 16, 16).T, (8, 1)))


def preprocess(inputs, min_ch=None):
    """Full-problem inputs -> (cfg, list of 8 per-core input maps)."""
    x = np.asarray(inputs["x"], np.float32)
    edge_attr = np.asarray(inputs["edge_attr"], np.float32)
    edge_attr3 = np.asarray(inputs["edge_attr3"], np.float32)
    edge_index = np.asarray(inputs["edge_index"], np.int64)
    edge_index3 = np.asarray(inputs["edge_index3"], np.int64)
    n, e, p = x.shape[0], edge_index.shape[1], edge_index3.shape[1]
    n_loc = n // N_CORES
    tile_nodes = min(512, n_loc)

    src, dst = edge_index[0], edge_index[1]
    deg = np.maximum(np.bincount(dst, minlength=n), 1).astype(np.float32)
    rdeg = (1.0 / deg).astype(np.float32)
    order = np.argsort(dst, kind="stable")
    dst_sorted = dst[order]

    # tile boundaries
    n_tile_tot = n // tile_nodes
    bounds = np.searchsorted(dst_sorted, np.arange(n_tile_tot + 1) * tile_nodes)
    counts = np.diff(bounds)
    ch_per_tile = int(np.ceil(counts.max() / 128))
    if min_ch is not None:
        ch_per_tile = max(ch_per_tile, min_ch)
    use_b2 = bool(np.any(np.asarray(inputs["b2"], np.float32)))
    cfg = Cfg(n, e, p, ch_per_tile, use_b2=use_b2)
    tiles_per_core = n_loc // tile_nodes

    Wn = np.asarray(inputs["Wn"], np.float32)
    bn = np.asarray(inputs["bn"], np.float32)
    We = np.asarray(inputs["We"], np.float32)
    be = np.asarray(inputs["be"], np.float32)
    W1 = np.asarray(inputs["W1"], np.float32)
    b1 = np.asarray(inputs["b1"], np.float32)
    W2 = np.asarray(inputs["W2"], np.float32)
    b2 = np.asarray(inputs["b2"], np.float32)
    conv_b = np.asarray(inputs["conv_b"], np.float32)
    Wih = np.asarray(inputs["Wih"], np.float32)
    Whh = np.asarray(inputs["Whh"], np.float32)
    bih = np.asarray(inputs["bih"], np.float32)
    bhh = np.asarray(inputs["bhh"], np.float32)
    Wlw = np.asarray(inputs["Wlw"], np.float32)
    Wlb = np.asarray(inputs["Wlb"], np.float32)
    gamma = np.asarray(inputs["gamma"], np.float32)
    beta = np.asarray(inputs["beta"], np.float32)

    xT = np.vstack([x.T, np.ones((1, n), np.float32)])
    Wn_ext = np.vstack([Wn, bn[None, :]])
    # permute W2 columns (i,o) -> (o,i) so the device-side contraction
    # over i is unit-stride innermost
    W2p = np.ascontiguousarray(
        W2.reshape(HID, DIM, DIM).transpose(0, 2, 1).reshape(HID, DIM * DIM))
    b2pf = np.ascontiguousarray(b2.reshape(DIM, DIM).T.reshape(1, DIM * DIM))
    conv_bc = np.tile(conv_b[None, :], (128, 1))
    WihT = np.ascontiguousarray(Wih.T)
    WhhT = np.ascontiguousarray(Whh.T)
    bsum = bih + bhh
    b_r = bsum[0:DIM, None].copy()
    b_z = bsum[DIM:2 * DIM, None].copy()
    bihn = bih[2 * DIM:, None].copy()
    bhhn = bhh[2 * DIM:, None].copy()

    shared = dict(
        Wn_ext=Wn_ext, We_in=We, W1_in=W1, W2=W2p,
        be_c=be[:, None], b1_c=b1[:, None],
        conv_bc=conv_bc,
        b_r=b_r, b_z=b_z, bihn=bihn, bhhn=bhhn,
        Wlw=Wlw, Wlb=Wlb.reshape(8, 1),
        gamma=gamma[None, :], beta=beta[None, :],
        epsv=np.concatenate([np.full(DIM, 4.0 * EPS, np.float32),
                             np.full(2 * DIM, EPS, np.float32)])[None, :],
    )
    shared = {k: np.ascontiguousarray(v, dtype=np.float32)
              for k, v in shared.items()}
    shared["b2p"] = np.ascontiguousarray(b2pf.astype(BF))
    shared["WihT"] = np.ascontiguousarray(WihT.astype(BF))
    shared["WhhT"] = np.ascontiguousarray(WhhT.astype(BF))

    p_loc = p // N_CORES
    in_maps = []
    pair_perms = []
    for core in range(N_CORES):
        gidx_tok = np.zeros(cfg.e_pad, np.int64)
        oh = np.zeros((128, cfg.chunks, tile_nodes), np.float32)
        ea_rows = np.zeros((cfg.e_pad, edge_attr.shape[1]), np.float32)
        for t in range(tiles_per_core):
            gt = core * tiles_per_core + t
            sel = order[bounds[gt]:bounds[gt + 1]]
            # sort the tile's edges by src so the gather walks ascending
            # addresses (HBM-friendly); the one-hot follows the same order
            sel = sel[np.argsort(src[sel], kind="stable")]
            cnt = sel.shape[0]
            base = t * cfg.ch_per_tile * 128
            gidx_tok[base:base + cnt] = src[sel]
            dl = (dst[sel] - gt * tile_nodes).astype(np.int64)
            ee = np.arange(cnt)
            oh[(base + ee) % 128, (base + ee) // 128, dl] = 1.0
            ea_rows[base:base + cnt] = edge_attr[sel]
        eaT_core = ea_rows.T
        nsub = n_loc // 128
        rdegc = np.ascontiguousarray(
            rdeg[core * n_loc:(core + 1) * n_loc].reshape(nsub, 128).T)
        sl = slice(core * p_loc, (core + 1) * p_loc)
        # sort the core's pairs by first index (ascending gather addresses);
        # postprocess applies the inverse permutation
        pp_ = np.argsort(edge_index3[0, sl], kind="stable")
        pair_perms.append(pp_)
        m = dict(shared)
        m.update(
            xTl=np.ascontiguousarray(xT[:, core * n_loc:(core + 1) * n_loc]),
            eaT=np.ascontiguousarray(eaT_core),
            gidx=_wrap16(gidx_tok),
            ohd=np.ascontiguousarray(
                oh.reshape(128, cfg.chunks * tile_nodes).astype(BF)),
            rdegc=rdegc,
            idx30=_wrap16(edge_index3[0, sl][pp_]),
            idx31=_wrap16(edge_index3[1, sl][pp_]),
            ea3T=np.ascontiguousarray(edge_attr3[sl][pp_].T.astype(BF)),
        )
        in_maps.append(m)
    cfg.pair_perms = pair_perms
    return cfg, in_maps


def postprocess(cfg: Cfg, results):
    out = np.empty(cfg.p, np.float32)
    for core in range(N_CORES):
        yc = results[core]["y"]            # [128, p_chunks]; pair j = g*128+p
        seg = out[core * cfg.p_loc:(core + 1) * cfg.p_loc]
        seg[cfg.pair_perms[core]] = yc.T.reshape(-1)
    return out


def kernel(**inputs):
    cfg, in_maps = preprocess(inputs)
    nc = _get_nc(cfg)
    res = bass_utils.run_bass_kernel_spmd(nc, in_maps,
                                          core_ids=list(range(N_CORES)))
    return postprocess(cfg, res.results)
